# revision 16
# baseline (speedup 1.0000x reference)
"""Trainium2 Bass kernel for 1D multi-scale deformable attention (v2).

Self-contained: builds the Bass/Tile program, shards the full inputs
data-parallel over N across 8 NeuronCores, runs via run_bass_kernel_spmd,
and returns the full (N, LQ, 256) output.

v2 vs baseline: bf16 value/gather/weighted-sum datapath (2x DVE modes,
half the gather traffic), raw-offset ix computation (offsets not
pre-divided by T), floor-after-min base computation (casts on (128,4)
instead of (128,128)), paired indirect gathers (2 calls/tile instead
of 4), paired tree reductions, engine-assignment knobs.

Algorithm per core (one batch element):
  value = vin @ W_val.T + b_val            -> padded natural layout (T', 256)
  offs  = q @ W_off.T + b_off              -> ix = ref*T - 0.5 + offs
  attn  = softmax(q @ W_attn.T + b_attn)   per (q, m) over 16 (l,p)
  bilinear + zero padding == sum_t relu(1 - |ix - t|) * V[t], t in [0, T)
  per (q,l): all-head window, base = clamp(floor(min over (m,p) of
  masked relu(ix))), static width W_l; indirect-DMA gathers W_l full
  value rows per query; u[m,j] = sum_p attn * relu(1 - |ix_p - (base+j)|)
  out[q, m*32+d] = sum_{l,j} u * G
"""
import os
import numpy as np
from contextlib import ExitStack

import concourse.bass as bass
import concourse.bacc as bacc
import concourse.tile as tile
from concourse import mybir
from concourse.masks import make_identity
from concourse.bass_utils import run_bass_kernel_spmd

f32 = mybir.dt.float32
bf16 = mybir.dt.bfloat16
i32 = mybir.dt.int32
ALU = mybir.AluOpType
ACT = mybir.ActivationFunctionType

# static problem config
LENS = (2048, 1024, 512, 256)
N, LQ, DM = 8, 2048, 256
M, L, P, DH = 8, 4, 4, 32
S = sum(LENS)                      # 3840
WCONF = (8, 10, 8, 10)             # per-level all-head window rows
PAIRS = ((0, 2), (1, 3))           # levels grouped by equal W
PAD = 12                           # zero rows after each level (>= max(W)-1)
LSTARTP = []
_s = 0
for _T in LENS:
    LSTARTP.append(_s)
    _s += _T + PAD
TPR = _s                           # 3888 padded rows total
NQT = LQ // 128                    # 16 query tiles
NVT = S // 128                     # 30 value tiles
BIG = 100000.0

# consts layout (one row, broadcast to 128 partitions at load)
C_TL = 0             # 4: T_l
C_TM1L = 4           # 4: T_l - 1
C_LST = 8            # 4: LSTARTP[l]
C_JROW = 12          # 16: j = 0..15
C_NEG1 = 28          # 1: -1.0
CW = 29


def _ap(base, dims, extra_offset=0):
    """Custom strided AP derived from a 2D (128, F) contiguous tile AP."""
    return bass.AP(
        tensor=base.tensor,
        offset=base.offset + extra_offset,
        ap=[list(base.ap[0])] + [[s, c] for s, c in dims],
    )


def build_program():
    DT = f32 if os.environ.get("DEFORM_DT") == "f32" else bf16
    prod_gps = os.environ.get("DEFORM_PROD_GPS", "13")     # levels on gpsimd
    tree_gps = os.environ.get("DEFORM_TREE_GPS", "")       # pair ids on gpsimd

    nc = bacc.Bacc("TRN2", target_bir_lowering=False, debug=False)

    q_d = nc.dram_tensor("q", [LQ, DM], f32, kind="ExternalInput")
    ref_d = nc.dram_tensor("ref", [LQ, L], f32, kind="ExternalInput")
    vin_d = nc.dram_tensor("vin", [S, DM], f32, kind="ExternalInput")
    wv_d = nc.dram_tensor("wv", [DM + 1, DM], f32, kind="ExternalInput")
    wof_d = nc.dram_tensor("wof", [DM + 1, M * L * P], f32, kind="ExternalInput")
    wat_d = nc.dram_tensor("wat", [DM + 1, M * L * P], f32, kind="ExternalInput")
    consts_d = nc.dram_tensor("consts", [1, CW], f32, kind="ExternalInput")
    out_d = nc.dram_tensor("out", [LQ, DM], f32, kind="ExternalOutput")

    with tile.TileContext(nc) as tc, ExitStack() as ctx:
        singles = ctx.enter_context(tc.tile_pool(name="singles", bufs=1))
        dram = ctx.enter_context(tc.tile_pool(name="dram", bufs=1, space="DRAM"))
        vpool = ctx.enter_context(tc.tile_pool(name="vpool", bufs=3))
        psum = ctx.enter_context(tc.tile_pool(name="psum", bufs=2, space="PSUM"))
        qpool = ctx.enter_context(tc.tile_pool(name="qpool", bufs=2))
        gpool = ctx.enter_context(tc.tile_pool(name="gpool", bufs=2))
        spool = ctx.enter_context(tc.tile_pool(name="spool", bufs=2))

        # ---- constants / weights (loaded once)
        ident = singles.tile([128, 128], f32)
        make_identity(nc, ident[:])
        ones_row = singles.tile([1, 128], DT)
        nc.vector.memset(ones_row[:], 1.0)
        consts = singles.tile([128, CW], f32)
        nc.sync.dma_start(
            out=consts[:],
            in_=bass.AP(tensor=consts_d[:].tensor, offset=0,
                        ap=[[0, 128], [1, CW]]),
        )
        # load f32 weights then cast once to DT
        wtmp = singles.tile([128, DM], f32)
        wv0 = singles.tile([128, DM], DT)
        wv1 = singles.tile([128, DM], DT)
        wvb = singles.tile([1, DM], DT)
        wof0 = singles.tile([128, 128], DT)
        wof1 = singles.tile([128, 128], DT)
        wofb = singles.tile([1, 128], DT)
        wat0 = singles.tile([128, 128], DT)
        wat1 = singles.tile([128, 128], DT)
        watb = singles.tile([1, 128], DT)
        wtmpb = singles.tile([1, DM], f32)
        for dst, src_d, r0, cols in (
                (wv0, wv_d, 0, DM), (wv1, wv_d, 128, DM),
                (wof0, wof_d, 0, 128), (wof1, wof_d, 128, 128),
                (wat0, wat_d, 0, 128), (wat1, wat_d, 128, 128)):
            nc.sync.dma_start(out=wtmp[:, :cols], in_=src_d[r0:r0 + 128, :])
            nc.scalar.copy(out=dst[:], in_=wtmp[:, :cols])
        for dst, src_d, cols in ((wvb, wv_d, DM), (wofb, wof_d, 128),
                                 (watb, wat_d, 128)):
            nc.sync.dma_start(out=wtmpb[:, :cols], in_=src_d[256:257, :])
            nc.scalar.copy(out=dst[:], in_=wtmpb[:, :cols])

        # ---- value scratch: natural padded rows (TPR, 256) in DT
        vp = dram.tile([TPR, DM], DT)
        zt = singles.tile([128, DM], DT)
        nc.vector.memset(zt[:], 0.0)
        for l, T in enumerate(LENS):
            nc.sync.dma_start(
                out=vp[:][LSTARTP[l] + T:LSTARTP[l] + T + PAD, :],
                in_=zt[:PAD, :])

        # ---- phase A: value projection into vp
        for tt in range(NVT):
            vt = vpool.tile([128, DM], f32, tag="vt")
            nc.sync.dma_start(out=vt[:], in_=vin_d[tt * 128:(tt + 1) * 128, :])
            ps0 = psum.tile([128, 128], f32, tag="tr")
            ps1 = psum.tile([128, 128], f32, tag="tr")
            nc.tensor.transpose(out=ps0[:], in_=vt[:, 0:128], identity=ident[:])
            nc.tensor.transpose(out=ps1[:], in_=vt[:, 128:256], identity=ident[:])
            vT0 = vpool.tile([128, 128], DT, tag="vT")
            vT1 = vpool.tile([128, 128], DT, tag="vT")
            nc.vector.tensor_copy(out=vT0[:], in_=ps0[:])
            nc.scalar.copy(out=vT1[:], in_=ps1[:])
            pv = psum.tile([128, DM], f32, tag="mm")
            nc.tensor.matmul(out=pv[:], lhsT=vT0[:], rhs=wv0[:], start=True, stop=False)
            nc.tensor.matmul(out=pv[:], lhsT=vT1[:], rhs=wv1[:], start=False, stop=False)
            nc.tensor.matmul(out=pv[:], lhsT=ones_row[:], rhs=wvb[:], start=False, stop=True)
            st = vpool.tile([128, DM], DT, tag="st")
            nc.scalar.copy(out=st[:], in_=pv[:])
            row0 = tt * 128
            acc = 0
            for li, T in enumerate(LENS):
                if row0 < acc + T:
                    l, trel = li, row0 - acc
                    break
                acc += T
            dst = LSTARTP[l] + trel
            nc.sync.dma_start(out=vp[:][dst:dst + 128, :], in_=st[:])

        # ---- phase B: interleaved B1 (prefix math) / B2 (gather + weighted
        # sum) with a tile lag so B2 of tile qt-LAG pipelines behind B1 of qt.
        # B1 parks u-weights (UB8s/UB10s, layout [j][lv][m] per tile) + IDXs.
        UB8s = singles.tile([128, NQT * 2 * M * 8], DT)
        UB10s = singles.tile([128, NQT * 2 * M * 10], DT)
        IDXs = singles.tile([128, NQT * L], i32)

        # NOTE: cce_op=mult on DMA is rejected by the neuronxcc BIR verifier
        # (assertDMACopySupportedCceOp) — the gather+multiply fusion only
        # works in CoreSim.  Default to the TT fallback path.
        use_cce = os.environ.get("DEFORM_CCE", "0") == "1"
        ue_eng_s = os.environ.get("DEFORM_UE_ENG", "sgss")
        LAG = int(os.environ.get("DEFORM_LAG", "2"))

        def eng_of(c):
            return {"s": nc.scalar, "g": nc.gpsimd}.get(c, nc.vector)

        def emit_b1(qt):
            qtile = qpool.tile([128, DM], f32, tag="qtile")
            reft = qpool.tile([128, L], f32, tag="reft")
            nc.sync.dma_start(out=qtile[:], in_=q_d[qt * 128:(qt + 1) * 128, :])
            nc.sync.dma_start(out=reft[:], in_=ref_d[qt * 128:(qt + 1) * 128, :])

            psq0 = psum.tile([128, 128], f32, tag="tr")
            psq1 = psum.tile([128, 128], f32, tag="tr")
            nc.tensor.transpose(out=psq0[:], in_=qtile[:, 0:128], identity=ident[:])
            nc.tensor.transpose(out=psq1[:], in_=qtile[:, 128:256], identity=ident[:])
            qT0 = qpool.tile([128, 128], DT, tag="qT")
            qT1 = qpool.tile([128, 128], DT, tag="qT")
            nc.scalar.copy(out=qT0[:], in_=psq0[:])
            nc.scalar.copy(out=qT1[:], in_=psq1[:])

            offp = psum.tile([128, 128], f32, tag="mm")
            nc.tensor.matmul(out=offp[:], lhsT=qT0[:], rhs=wof0[:], start=True, stop=False)
            nc.tensor.matmul(out=offp[:], lhsT=qT1[:], rhs=wof1[:], start=False, stop=False)
            nc.tensor.matmul(out=offp[:], lhsT=ones_row[:], rhs=wofb[:], start=False, stop=True)
            attp = psum.tile([128, 128], f32, tag="mm")
            nc.tensor.matmul(out=attp[:], lhsT=qT0[:], rhs=wat0[:], start=True, stop=False)
            nc.tensor.matmul(out=attp[:], lhsT=qT1[:], rhs=wat1[:], start=False, stop=False)
            nc.tensor.matmul(out=attp[:], lhsT=ones_row[:], rhs=watb[:], start=False, stop=True)

            # softmax numerator (no max-sub: |logits| < ~4); normalization
            # folded into U via rr
            E = qpool.tile([128, 128], f32, tag="E")
            nc.scalar.activation(out=E[:], in_=attp[:], func=ACT.Exp)
            sm = qpool.tile([128, M], f32, tag="sm")
            nc.vector.tensor_reduce(out=sm[:], in_=E[:].rearrange("p (m k) -> p m k", m=M),
                                    axis=mybir.AxisListType.X, op=ALU.add)
            rr = qpool.tile([128, M], f32, tag="rr")
            nc.vector.reciprocal(out=rr[:], in_=sm[:])

            # ix = ref*T - 0.5 + offs   (offs raw, not pre-divided by T)
            REFTS = qpool.tile([128, L], f32, tag="REFTS")
            nc.vector.tensor_tensor(out=REFTS[:], in0=reft[:],
                                    in1=consts[:, C_TL:C_TL + L], op=ALU.mult)
            nc.vector.tensor_scalar(out=REFTS[:], in0=REFTS[:], scalar1=0.5,
                                    scalar2=None, op0=ALU.subtract)
            IX = qpool.tile([128, 128], f32, tag="IX")
            nc.vector.tensor_tensor(out=IX[:], in0=offp[:],
                                    in1=_ap(REFTS[:], [[0, M], [1, L], [0, P]]),
                                    op=ALU.add)

            # base per (q,l): clamp(floor(min over (m,p) of masked relu(ix)))
            REL = qpool.tile([128, 128], f32, tag="REL")
            nc.scalar.activation(out=REL[:], in_=IX[:], func=ACT.Relu)
            MSK = qpool.tile([128, 128], f32, tag="MSK")
            nc.scalar.activation(out=MSK[:], in_=IX[:], func=ACT.Relu,
                                 bias=consts[:, C_NEG1:C_NEG1 + 1], scale=-1.0)
            MSKs = qpool.tile([128, 128], f32, tag="MSKs")
            nc.vector.tensor_scalar(out=MSKs[:], in0=MSK[:], scalar1=1e13,
                                    scalar2=BIG, op0=ALU.mult, op1=ALU.min)
            NLF = qpool.tile([128, 128], f32, tag="NLF")
            nc.vector.tensor_tensor(out=NLF[:], in0=REL[:], in1=MSKs[:], op=ALU.add)
            BMIN = qpool.tile([128, L], f32, tag="BMIN")
            nc.vector.tensor_reduce(out=BMIN[:],
                                    in_=_ap(NLF[:], [[P, L], [P * L, M], [1, P]]),
                                    axis=mybir.AxisListType.XY, op=ALU.min)
            # floor on the (128, L) mins (floor commutes with min)
            FLI = qpool.tile([128, L], i32, tag="FLI")
            nc.vector.tensor_copy(out=FLI[:], in_=BMIN[:])
            FLR = qpool.tile([128, L], f32, tag="FLR")
            nc.vector.tensor_copy(out=FLR[:], in_=FLI[:])
            GT = qpool.tile([128, L], f32, tag="GT")
            nc.vector.tensor_tensor(out=GT[:], in0=FLR[:], in1=BMIN[:], op=ALU.is_gt)
            FL = qpool.tile([128, L], f32, tag="FL")
            nc.vector.tensor_tensor(out=FL[:], in0=FLR[:], in1=GT[:], op=ALU.subtract)
            BASEL = qpool.tile([128, L], f32, tag="BASEL")
            nc.vector.tensor_tensor(out=BASEL[:], in0=FL[:],
                                    in1=consts[:, C_TM1L:C_TM1L + L], op=ALU.min)
            # IDX columns permuted to pair order (l=0,2,1,3); park in slot qt
            IDXF = qpool.tile([128, L], f32, tag="IDXF")
            nc.vector.tensor_tensor(out=_ap(IDXF[:], [[2, 2], [1, 2]]),
                                    in0=_ap(BASEL[:], [[1, 2], [2, 2]]),
                                    in1=_ap(consts[:], [[1, 2], [2, 2]],
                                            extra_offset=C_LST),
                                    op=ALU.add)
            nc.vector.tensor_copy(out=IDXs[:, qt * L:(qt + 1) * L], in_=IDXF[:])

            # z = ix - base (all-head base per (q,l))
            Z = qpool.tile([128, 128], f32, tag="Z")
            nc.vector.tensor_tensor(out=Z[:], in0=IX[:],
                                    in1=_ap(BASEL[:], [[0, M], [1, L], [0, P]]),
                                    op=ALU.subtract)

            # u[m,j] weights per pair of levels
            D8 = spool.tile([128, 2 * M * P * 8], f32, tag="D8")
            D10 = spool.tile([128, 2 * M * P * 10], f32, tag="D10")
            for pi, pair in enumerate(PAIRS):
                W = WCONF[pair[0]]
                Dt = (D8, D10)[pi]
                UBslot = (UB8s, UB10s)[pi]
                blk = M * P * W
                for lv, l in enumerate(pair):
                    nc.vector.tensor_tensor(
                        out=_ap(Dt[:], [[P * W, M], [W, P], [1, W]],
                                extra_offset=lv * blk),
                        in0=_ap(Z[:], [[2 * M, M], [1, P], [0, W]],
                                extra_offset=l * P),
                        in1=_ap(consts[:], [[0, M], [0, P], [1, W]],
                                extra_offset=C_JROW),
                        op=ALU.subtract)
                # tent = relu(1 - |d|), both levels at once on scalar engine
                nc.scalar.activation(out=Dt[:], in_=Dt[:], func=ACT.Abs)
                nc.scalar.activation(out=Dt[:], in_=Dt[:], func=ACT.Relu,
                                     bias=1.0, scale=-1.0)
                HAt = spool.tile([128, 2 * M * P * 10], f32, tag=f"HA{pi}")
                for lv, l in enumerate(pair):
                    nc.vector.tensor_tensor(
                        out=_ap(HAt[:], [[P * W, M], [W, P], [1, W]],
                                extra_offset=lv * blk),
                        in0=_ap(Dt[:], [[P * W, M], [W, P], [1, W]],
                                extra_offset=lv * blk),
                        in1=_ap(E[:], [[2 * M, M], [1, P], [0, W]],
                                extra_offset=l * P),
                        op=ALU.mult)
                # sum over p (both levels at once): (lv,m) fused dim of 16
                U2 = spool.tile([128, 2 * M * 2 * 10], f32, tag=f"U2_{pi}")
                nc.vector.tensor_tensor(
                    out=_ap(U2[:], [[2 * W, 2 * M], [W, 2], [1, W]]),
                    in0=_ap(HAt[:], [[P * W, 2 * M], [2 * W, 2], [1, W]]),
                    in1=_ap(HAt[:], [[P * W, 2 * M], [2 * W, 2], [1, W]],
                            extra_offset=W),
                    op=ALU.add)
                U = spool.tile([128, 2 * M * 10], f32, tag=f"U_{pi}")
                nc.vector.tensor_tensor(
                    out=_ap(U[:], [[W, 2 * M], [1, W]]),
                    in0=_ap(U2[:], [[2 * W, 2 * M], [1, W]]),
                    in1=_ap(U2[:], [[2 * W, 2 * M], [1, W]], extra_offset=W),
                    op=ALU.add)
                # fold softmax normalization into u; cast to DT; park in
                # slot qt with layout [j][lv][m] (j-major).
                nc.vector.tensor_tensor(
                    out=_ap(UBslot[:], [[1, 2 * M], [2 * M, W]],
                            extra_offset=qt * 2 * M * W),
                    in0=_ap(U[:], [[W, 2 * M], [1, W]]),
                    in1=_ap(rr[:], [[0, 2], [1, M], [0, W]]),
                    op=ALU.mult)

        def emit_b2(qt):
            # PR[lv][j][m][d] per pair.  CCE path: expand u into PR, then the
            # indirect gather multiplies V rows in on the DMA compute engine.
            PR8 = spool.tile([128, 2 * 8 * M * DH], DT, tag="PR8")
            PR10 = spool.tile([128, 2 * 10 * M * DH], DT, tag="PR10")
            if use_cce:
                for pi, pair in enumerate(PAIRS):
                    W = WCONF[pair[0]]
                    PRt = (PR8, PR10)[pi]
                    UBslot = (UB8s, UB10s)[pi]
                    for lv, l in enumerate(pair):
                        ue = eng_of(ue_eng_s[l])
                        dst = _ap(PRt[:], [[M * DH, W], [DH, M], [1, DH]],
                                  extra_offset=lv * W * M * DH)
                        src = _ap(UBslot[:], [[2 * M, W], [1, M], [0, DH]],
                                  extra_offset=qt * 2 * M * W + lv * M)
                        if ue is nc.scalar:
                            nc.scalar.copy(out=dst, in_=src)
                        else:
                            ue.tensor_copy(out=dst, in_=src)
                for pi, pair in enumerate(PAIRS):
                    W = WCONF[pair[0]]
                    PRt = (PR8, PR10)[pi]
                    for lv, l in enumerate(pair):
                        nc.gpsimd.indirect_dma_start(
                            out=PRt[:, lv * W * DM:(lv + 1) * W * DM],
                            out_offset=None,
                            in_=vp[:],
                            in_offset=bass.IndirectOffsetOnAxis(
                                ap=IDXs[:, qt * L + 2 * pi + lv:
                                        qt * L + 2 * pi + lv + 1], axis=0),
                            oob_is_err=False,
                            compute_op=ALU.mult,
                        )
            else:
                GT8 = gpool.tile([128, 2 * 8 * DM], DT, tag="G8")
                GT10 = gpool.tile([128, 2 * 10 * DM], DT, tag="G10")
                for pi, (Gt, W) in enumerate(((GT8, 8), (GT10, 10))):
                    for lv in range(2):
                        nc.gpsimd.indirect_dma_start(
                            out=Gt[:, lv * W * DM:(lv + 1) * W * DM],
                            out_offset=None,
                            in_=vp[:],
                            in_offset=bass.IndirectOffsetOnAxis(
                                ap=IDXs[:, qt * L + 2 * pi + lv:
                                        qt * L + 2 * pi + lv + 1], axis=0),
                            oob_is_err=False,
                        )
                for pi, pair in enumerate(PAIRS):
                    W = WCONF[pair[0]]
                    Gt = (GT8, GT10)[pi]
                    PRt = (PR8, PR10)[pi]
                    UBslot = (UB8s, UB10s)[pi]
                    for lv, l in enumerate(pair):
                        eng = nc.gpsimd if str(l) in prod_gps else nc.vector
                        eng.tensor_tensor(
                            out=_ap(PRt[:], [[M * DH, W], [DH, M], [1, DH]],
                                    extra_offset=lv * W * M * DH),
                            in0=_ap(Gt[:], [[M * DH, W], [DH, M], [1, DH]],
                                    extra_offset=lv * W * DM),
                            in1=_ap(UBslot[:], [[2 * M, W], [1, M], [0, DH]],
                                    extra_offset=qt * 2 * M * W + lv * M),
                            op=ALU.mult)

            # j-tree sums per pair: contiguous halves within each lv block
            for pi, pair in enumerate(PAIRS):
                W = WCONF[pair[0]]
                PRt = (PR8, PR10)[pi]
                eng = nc.gpsimd if str(pi) in tree_gps else nc.vector
                if W == 10:     # fold j in {8,9} onto {0,1} first
                    eng.tensor_tensor(
                        out=_ap(PRt[:], [[W * M * DH, 2], [1, 2 * M * DH]]),
                        in0=_ap(PRt[:], [[W * M * DH, 2], [1, 2 * M * DH]]),
                        in1=_ap(PRt[:], [[W * M * DH, 2], [1, 2 * M * DH]],
                                extra_offset=8 * M * DH),
                        op=ALU.add)
                w = 8
                while w > 1:
                    h = w // 2
                    eng.tensor_tensor(
                        out=_ap(PRt[:], [[W * M * DH, 2], [1, h * M * DH]]),
                        in0=_ap(PRt[:], [[W * M * DH, 2], [1, h * M * DH]]),
                        in1=_ap(PRt[:], [[W * M * DH, 2], [1, h * M * DH]],
                                extra_offset=h * M * DH),
                        op=ALU.add)
                    w = h

            # sum levels: OUT = (PR8_l0 + PR8_l2) + (PR10_l1 + PR10_l3)
            T8 = spool.tile([128, DM], f32, tag="T8")
            nc.vector.tensor_tensor(
                out=T8[:], in0=PR8[:, 0:DM],
                in1=_ap(PR8[:], [[1, DM]], extra_offset=8 * M * DH),
                op=ALU.add)
            T10 = spool.tile([128, DM], f32, tag="T10")
            nc.vector.tensor_tensor(
                out=T10[:], in0=PR10[:, 0:DM],
                in1=_ap(PR10[:], [[1, DM]], extra_offset=10 * M * DH),
                op=ALU.add)
            OUTT = spool.tile([128, DM], f32, tag="OUTT")
            nc.vector.tensor_tensor(out=OUTT[:], in0=T8[:], in1=T10[:], op=ALU.add)
            nc.sync.dma_start(out=out_d[qt * 128:(qt + 1) * 128, :], in_=OUTT[:])

        rep = max(1, int(os.environ.get("DEFORM_REPEAT", "1")))
        for _ in range(rep):
            for i in range(NQT + LAG):
                if i < NQT:
                    emit_b1(i)
                if i >= LAG:
                    emit_b2(i - LAG)

    nc.compile()
    return nc


def host_prep(inputs):
    """Build per-core in_maps from full inputs."""
    q = np.ascontiguousarray(inputs["query"], np.float32)
    ref = np.ascontiguousarray(np.asarray(inputs["reference_points"])[..., 0], np.float32)
    vin = np.ascontiguousarray(inputs["input_flatten"], np.float32)
    W_val = np.asarray(inputs["W_val"], np.float32)
    b_val = np.asarray(inputs["b_val"], np.float32)
    W_off = np.asarray(inputs["W_off"], np.float32)
    b_off = np.asarray(inputs["b_off"], np.float32)
    W_attn = np.asarray(inputs["W_attn"], np.float32)
    b_attn = np.asarray(inputs["b_attn"], np.float32)

    wv = np.concatenate([W_val.T, b_val[None, :]], 0)
    wof = np.concatenate([W_off.T, b_off[None, :]], 0)
    wat = np.concatenate([W_attn.T, b_attn[None, :]], 0)

    consts = np.zeros((1, CW), np.float32)
    for l in range(L):
        consts[0, C_TL + l] = LENS[l]
        consts[0, C_TM1L + l] = LENS[l] - 1
        consts[0, C_LST + l] = LSTARTP[l]
    consts[0, C_JROW:C_JROW + 16] = np.arange(16, dtype=np.float32)
    consts[0, C_NEG1] = -1.0

    shared = {"wv": np.ascontiguousarray(wv), "wof": np.ascontiguousarray(wof),
              "wat": np.ascontiguousarray(wat), "consts": consts}
    return [
        {"q": q[n], "ref": ref[n], "vin": vin[n], **shared}
        for n in range(N)
    ]


_NC_CACHE = None


def kernel(**inputs) -> np.ndarray:
    global _NC_CACHE
    if _NC_CACHE is None:
        _NC_CACHE = build_program()
    nc = _NC_CACHE
    in_maps = host_prep(inputs)
    res = run_bass_kernel_spmd(nc, in_maps, list(range(N)))
    return np.stack([res.results[n]["out"] for n in range(N)]).astype(np.float32)


if __name__ == "__main__":
    d = np.load("/root/problem/cached_io.npz")
    inp = {k: d[k] for k in ["query", "reference_points", "input_flatten",
                             "input_temporal_lens", "input_level_start_index",
                             "W_val", "b_val", "W_off", "b_off", "W_attn", "b_attn"]}
    out = kernel(**inp)
    ref = d["ref_out"]
    err = np.abs(out - ref).max()
    print("absmax err:", err, "scale:", np.abs(ref).max(),
          "rel:", err / np.abs(ref).max())


# revision 18
# speedup vs baseline: 1.1110x; 1.1110x over previous
"""Trainium2 Bass kernel for 1D multi-scale deformable attention (v2).

Self-contained: builds the Bass/Tile program, shards the full inputs
data-parallel over N across 8 NeuronCores, runs via run_bass_kernel_spmd,
and returns the full (N, LQ, 256) output.

v2 vs baseline: bf16 value/gather/weighted-sum datapath (2x DVE modes,
half the gather traffic), raw-offset ix computation (offsets not
pre-divided by T), floor-after-min base computation (casts on (128,4)
instead of (128,128)), paired indirect gathers (2 calls/tile instead
of 4), paired tree reductions, engine-assignment knobs.

Algorithm per core (one batch element):
  value = vin @ W_val.T + b_val            -> padded natural layout (T', 256)
  offs  = q @ W_off.T + b_off              -> ix = ref*T - 0.5 + offs
  attn  = softmax(q @ W_attn.T + b_attn)   per (q, m) over 16 (l,p)
  bilinear + zero padding == sum_t relu(1 - |ix - t|) * V[t], t in [0, T)
  per (q,l): all-head window, base = clamp(floor(min over (m,p) of
  masked relu(ix))), static width W_l; indirect-DMA gathers W_l full
  value rows per query; u[m,j] = sum_p attn * relu(1 - |ix_p - (base+j)|)
  out[q, m*32+d] = sum_{l,j} u * G
"""
import os
import numpy as np
from contextlib import ExitStack

import concourse.bass as bass
import concourse.bacc as bacc
import concourse.tile as tile
from concourse import mybir
from concourse.masks import make_identity
from concourse.bass_utils import run_bass_kernel_spmd

f32 = mybir.dt.float32
bf16 = mybir.dt.bfloat16
i32 = mybir.dt.int32
ALU = mybir.AluOpType
ACT = mybir.ActivationFunctionType

# static problem config
LENS = (2048, 1024, 512, 256)
N, LQ, DM = 8, 2048, 256
M, L, P, DH = 8, 4, 4, 32
S = sum(LENS)                      # 3840
WCONF = (8, 10, 8, 10)             # per-level all-head window rows
PAIRS = ((0, 2), (1, 3))           # levels grouped by equal W
PAD = 12                           # zero rows after each level (>= max(W)-1)
LSTARTP = []
_s = 0
for _T in LENS:
    LSTARTP.append(_s)
    _s += _T + PAD
TPR = _s                           # 3888 padded rows total
NQT = LQ // 128                    # 16 query tiles
NVT = S // 128                     # 30 value tiles
BIG = 100000.0

# consts layout (one row, broadcast to 128 partitions at load)
C_TL = 0             # 4: T_l
C_TM1L = 4           # 4: T_l - 1
C_LST = 8            # 4: LSTARTP[l]
C_JROW = 12          # 16: j = 0..15
C_NEG1 = 28          # 1: -1.0
CW = 29


def _ap(base, dims, extra_offset=0):
    """Custom strided AP derived from a 2D (128, F) contiguous tile AP."""
    return bass.AP(
        tensor=base.tensor,
        offset=base.offset + extra_offset,
        ap=[list(base.ap[0])] + [[s, c] for s, c in dims],
    )


def build_program():
    DT = f32 if os.environ.get("DEFORM_DT") == "f32" else bf16
    prod_gps = os.environ.get("DEFORM_PROD_GPS", "3")      # levels on gpsimd
    tree_gps = os.environ.get("DEFORM_TREE_GPS", "")       # pair ids on gpsimd

    nc = bacc.Bacc("TRN2", target_bir_lowering=False, debug=False)

    q_d = nc.dram_tensor("q", [LQ, DM], f32, kind="ExternalInput")
    ref_d = nc.dram_tensor("ref", [LQ, L], f32, kind="ExternalInput")
    vin_d = nc.dram_tensor("vin", [S, DM], f32, kind="ExternalInput")
    wv_d = nc.dram_tensor("wv", [DM + 1, DM], f32, kind="ExternalInput")
    wof_d = nc.dram_tensor("wof", [DM + 1, M * L * P], f32, kind="ExternalInput")
    wat_d = nc.dram_tensor("wat", [DM + 1, M * L * P], f32, kind="ExternalInput")
    consts_d = nc.dram_tensor("consts", [1, CW], f32, kind="ExternalInput")
    out_d = nc.dram_tensor("out", [LQ, DM], f32, kind="ExternalOutput")

    with tile.TileContext(nc) as tc, ExitStack() as ctx:
        singles = ctx.enter_context(tc.tile_pool(name="singles", bufs=1))
        dram = ctx.enter_context(tc.tile_pool(name="dram", bufs=1, space="DRAM"))
        vpool = ctx.enter_context(tc.tile_pool(name="vpool", bufs=3))
        psum = ctx.enter_context(tc.tile_pool(name="psum", bufs=2, space="PSUM"))
        qpool = ctx.enter_context(tc.tile_pool(name="qpool", bufs=2))
        gpool = ctx.enter_context(tc.tile_pool(name="gpool", bufs=3))
        spool = ctx.enter_context(tc.tile_pool(name="spool", bufs=2))

        # ---- constants / weights (loaded once)
        ident = singles.tile([128, 128], f32)
        make_identity(nc, ident[:])
        ones_row = singles.tile([1, 128], DT)
        nc.vector.memset(ones_row[:], 1.0)
        consts = singles.tile([128, CW], f32)
        nc.sync.dma_start(
            out=consts[:],
            in_=bass.AP(tensor=consts_d[:].tensor, offset=0,
                        ap=[[0, 128], [1, CW]]),
        )
        # load f32 weights then cast once to DT
        wtmp = singles.tile([128, DM], f32)
        wv0 = singles.tile([128, DM], DT)
        wv1 = singles.tile([128, DM], DT)
        wvb = singles.tile([1, DM], DT)
        wof0 = singles.tile([128, 128], DT)
        wof1 = singles.tile([128, 128], DT)
        wofb = singles.tile([1, 128], DT)
        wat0 = singles.tile([128, 128], DT)
        wat1 = singles.tile([128, 128], DT)
        watb = singles.tile([1, 128], DT)
        wtmpb = singles.tile([1, DM], f32)
        for dst, src_d, r0, cols in (
                (wv0, wv_d, 0, DM), (wv1, wv_d, 128, DM),
                (wof0, wof_d, 0, 128), (wof1, wof_d, 128, 128),
                (wat0, wat_d, 0, 128), (wat1, wat_d, 128, 128)):
            nc.sync.dma_start(out=wtmp[:, :cols], in_=src_d[r0:r0 + 128, :])
            nc.scalar.copy(out=dst[:], in_=wtmp[:, :cols])
        for dst, src_d, cols in ((wvb, wv_d, DM), (wofb, wof_d, 128),
                                 (watb, wat_d, 128)):
            nc.sync.dma_start(out=wtmpb[:, :cols], in_=src_d[256:257, :])
            nc.scalar.copy(out=dst[:], in_=wtmpb[:, :cols])

        # ---- value scratch: natural padded rows (TPR, 256) in DT
        vp = dram.tile([TPR, DM], DT)
        zt = singles.tile([128, DM], DT)
        nc.vector.memset(zt[:], 0.0)
        for l, T in enumerate(LENS):
            nc.sync.dma_start(
                out=vp[:][LSTARTP[l] + T:LSTARTP[l] + T + PAD, :],
                in_=zt[:PAD, :])

        # ---- phase A: value projection into vp
        for tt in range(NVT):
            vt = vpool.tile([128, DM], f32, tag="vt")
            nc.sync.dma_start(out=vt[:], in_=vin_d[tt * 128:(tt + 1) * 128, :])
            ps0 = psum.tile([128, 128], f32, tag="tr")
            ps1 = psum.tile([128, 128], f32, tag="tr")
            nc.tensor.transpose(out=ps0[:], in_=vt[:, 0:128], identity=ident[:])
            nc.tensor.transpose(out=ps1[:], in_=vt[:, 128:256], identity=ident[:])
            vT0 = vpool.tile([128, 128], DT, tag="vT")
            vT1 = vpool.tile([128, 128], DT, tag="vT")
            nc.vector.tensor_copy(out=vT0[:], in_=ps0[:])
            nc.scalar.copy(out=vT1[:], in_=ps1[:])
            pv = psum.tile([128, DM], f32, tag="mm")
            nc.tensor.matmul(out=pv[:], lhsT=vT0[:], rhs=wv0[:], start=True, stop=False)
            nc.tensor.matmul(out=pv[:], lhsT=vT1[:], rhs=wv1[:], start=False, stop=False)
            nc.tensor.matmul(out=pv[:], lhsT=ones_row[:], rhs=wvb[:], start=False, stop=True)
            st = vpool.tile([128, DM], DT, tag="st")
            nc.scalar.copy(out=st[:], in_=pv[:])
            row0 = tt * 128
            acc = 0
            for li, T in enumerate(LENS):
                if row0 < acc + T:
                    l, trel = li, row0 - acc
                    break
                acc += T
            dst = LSTARTP[l] + trel
            nc.sync.dma_start(out=vp[:][dst:dst + 128, :], in_=st[:])

        # ---- phase B: interleaved B1 (prefix math) / B2 (gather + weighted
        # sum) with a tile lag so B2 of tile qt-LAG pipelines behind B1 of qt.
        # B1 parks u-weights (UB8s/UB10s, layout [j][lv][m] per tile) + IDXs.
        UB8s = singles.tile([128, NQT * 2 * M * 8], DT)
        UB10s = singles.tile([128, NQT * 2 * M * 10], DT)
        IDXs = singles.tile([128, NQT * L], i32)

        # NOTE: cce_op=mult on DMA is rejected by the neuronxcc BIR verifier
        # (assertDMACopySupportedCceOp) — the gather+multiply fusion only
        # works in CoreSim.  Default to the TT fallback path.
        use_cce = os.environ.get("DEFORM_CCE", "0") == "1"
        ue_eng_s = os.environ.get("DEFORM_UE_ENG", "sgss")
        LAG = int(os.environ.get("DEFORM_LAG", "2"))

        def eng_of(c):
            return {"s": nc.scalar, "g": nc.gpsimd}.get(c, nc.vector)

        def emit_b1(qt):
            qtile = qpool.tile([128, DM], f32, tag="qtile")
            reft = qpool.tile([128, L], f32, tag="reft")
            nc.sync.dma_start(out=qtile[:], in_=q_d[qt * 128:(qt + 1) * 128, :])
            nc.sync.dma_start(out=reft[:], in_=ref_d[qt * 128:(qt + 1) * 128, :])

            psq0 = psum.tile([128, 128], f32, tag="tr")
            psq1 = psum.tile([128, 128], f32, tag="tr")
            nc.tensor.transpose(out=psq0[:], in_=qtile[:, 0:128], identity=ident[:])
            nc.tensor.transpose(out=psq1[:], in_=qtile[:, 128:256], identity=ident[:])
            qT0 = qpool.tile([128, 128], DT, tag="qT")
            qT1 = qpool.tile([128, 128], DT, tag="qT")
            nc.scalar.copy(out=qT0[:], in_=psq0[:])
            nc.scalar.copy(out=qT1[:], in_=psq1[:])

            offp = psum.tile([128, 128], f32, tag="mm")
            nc.tensor.matmul(out=offp[:], lhsT=qT0[:], rhs=wof0[:], start=True, stop=False)
            nc.tensor.matmul(out=offp[:], lhsT=qT1[:], rhs=wof1[:], start=False, stop=False)
            nc.tensor.matmul(out=offp[:], lhsT=ones_row[:], rhs=wofb[:], start=False, stop=True)
            attp = psum.tile([128, 128], f32, tag="mm")
            nc.tensor.matmul(out=attp[:], lhsT=qT0[:], rhs=wat0[:], start=True, stop=False)
            nc.tensor.matmul(out=attp[:], lhsT=qT1[:], rhs=wat1[:], start=False, stop=False)
            nc.tensor.matmul(out=attp[:], lhsT=ones_row[:], rhs=watb[:], start=False, stop=True)

            # softmax numerator (no max-sub: |logits| < ~4); normalization
            # folded into U via rr
            E = qpool.tile([128, 128], f32, tag="E")
            nc.scalar.activation(out=E[:], in_=attp[:], func=ACT.Exp)
            sm = qpool.tile([128, M], f32, tag="sm")
            nc.vector.tensor_reduce(out=sm[:], in_=E[:].rearrange("p (m k) -> p m k", m=M),
                                    axis=mybir.AxisListType.X, op=ALU.add)
            rr = qpool.tile([128, M], f32, tag="rr")
            nc.vector.reciprocal(out=rr[:], in_=sm[:])

            # ix = ref*T - 0.5 + offs   (offs raw, not pre-divided by T)
            REFTS = qpool.tile([128, L], f32, tag="REFTS")
            nc.vector.tensor_tensor(out=REFTS[:], in0=reft[:],
                                    in1=consts[:, C_TL:C_TL + L], op=ALU.mult)
            nc.vector.tensor_scalar(out=REFTS[:], in0=REFTS[:], scalar1=0.5,
                                    scalar2=None, op0=ALU.subtract)
            IX = qpool.tile([128, 128], f32, tag="IX")
            nc.vector.tensor_tensor(out=IX[:], in0=offp[:],
                                    in1=_ap(REFTS[:], [[0, M], [1, L], [0, P]]),
                                    op=ALU.add)

            # base per (q,l): clamp(floor(min over (m,p) of masked relu(ix)))
            REL = qpool.tile([128, 128], f32, tag="REL")
            nc.scalar.activation(out=REL[:], in_=IX[:], func=ACT.Relu)
            MSK = qpool.tile([128, 128], f32, tag="MSK")
            nc.scalar.activation(out=MSK[:], in_=IX[:], func=ACT.Relu,
                                 bias=consts[:, C_NEG1:C_NEG1 + 1], scale=-1.0)
            MSKs = qpool.tile([128, 128], f32, tag="MSKs")
            nc.vector.tensor_scalar(out=MSKs[:], in0=MSK[:], scalar1=1e13,
                                    scalar2=BIG, op0=ALU.mult, op1=ALU.min)
            NLF = qpool.tile([128, 128], f32, tag="NLF")
            nc.vector.tensor_tensor(out=NLF[:], in0=REL[:], in1=MSKs[:], op=ALU.add)
            BMIN = qpool.tile([128, L], f32, tag="BMIN")
            nc.vector.tensor_reduce(out=BMIN[:],
                                    in_=_ap(NLF[:], [[P, L], [P * L, M], [1, P]]),
                                    axis=mybir.AxisListType.XY, op=ALU.min)
            # floor on the (128, L) mins (floor commutes with min)
            FLI = qpool.tile([128, L], i32, tag="FLI")
            nc.vector.tensor_copy(out=FLI[:], in_=BMIN[:])
            FLR = qpool.tile([128, L], f32, tag="FLR")
            nc.vector.tensor_copy(out=FLR[:], in_=FLI[:])
            GT = qpool.tile([128, L], f32, tag="GT")
            nc.vector.tensor_tensor(out=GT[:], in0=FLR[:], in1=BMIN[:], op=ALU.is_gt)
            FL = qpool.tile([128, L], f32, tag="FL")
            nc.vector.tensor_tensor(out=FL[:], in0=FLR[:], in1=GT[:], op=ALU.subtract)
            BASEL = qpool.tile([128, L], f32, tag="BASEL")
            nc.vector.tensor_tensor(out=BASEL[:], in0=FL[:],
                                    in1=consts[:, C_TM1L:C_TM1L + L], op=ALU.min)
            # IDX columns permuted to pair order (l=0,2,1,3); park in slot qt
            IDXF = qpool.tile([128, L], f32, tag="IDXF")
            nc.vector.tensor_tensor(out=_ap(IDXF[:], [[2, 2], [1, 2]]),
                                    in0=_ap(BASEL[:], [[1, 2], [2, 2]]),
                                    in1=_ap(consts[:], [[1, 2], [2, 2]],
                                            extra_offset=C_LST),
                                    op=ALU.add)
            nc.vector.tensor_copy(out=IDXs[:, qt * L:(qt + 1) * L], in_=IDXF[:])

            # z = ix - base (all-head base per (q,l))
            Z = qpool.tile([128, 128], f32, tag="Z")
            nc.vector.tensor_tensor(out=Z[:], in0=IX[:],
                                    in1=_ap(BASEL[:], [[0, M], [1, L], [0, P]]),
                                    op=ALU.subtract)

            # u[m,j] weights per pair of levels
            D8 = spool.tile([128, 2 * M * P * 8], f32, tag="D8")
            D10 = spool.tile([128, 2 * M * P * 10], f32, tag="D10")
            for pi, pair in enumerate(PAIRS):
                W = WCONF[pair[0]]
                Dt = (D8, D10)[pi]
                UBslot = (UB8s, UB10s)[pi]
                blk = M * P * W
                for lv, l in enumerate(pair):
                    nc.vector.tensor_tensor(
                        out=_ap(Dt[:], [[P * W, M], [W, P], [1, W]],
                                extra_offset=lv * blk),
                        in0=_ap(Z[:], [[2 * M, M], [1, P], [0, W]],
                                extra_offset=l * P),
                        in1=_ap(consts[:], [[0, M], [0, P], [1, W]],
                                extra_offset=C_JROW),
                        op=ALU.subtract)
                # tent = relu(1 - |d|), both levels at once on scalar engine
                nc.scalar.activation(out=Dt[:], in_=Dt[:], func=ACT.Abs)
                nc.scalar.activation(out=Dt[:], in_=Dt[:], func=ACT.Relu,
                                     bias=1.0, scale=-1.0)
                HAt = spool.tile([128, 2 * M * P * 10], f32, tag=f"HA{pi}")
                for lv, l in enumerate(pair):
                    nc.vector.tensor_tensor(
                        out=_ap(HAt[:], [[P * W, M], [W, P], [1, W]],
                                extra_offset=lv * blk),
                        in0=_ap(Dt[:], [[P * W, M], [W, P], [1, W]],
                                extra_offset=lv * blk),
                        in1=_ap(E[:], [[2 * M, M], [1, P], [0, W]],
                                extra_offset=l * P),
                        op=ALU.mult)
                # sum over p (both levels at once): (lv,m) fused dim of 16
                U2 = spool.tile([128, 2 * M * 2 * 10], f32, tag=f"U2_{pi}")
                nc.vector.tensor_tensor(
                    out=_ap(U2[:], [[2 * W, 2 * M], [W, 2], [1, W]]),
                    in0=_ap(HAt[:], [[P * W, 2 * M], [2 * W, 2], [1, W]]),
                    in1=_ap(HAt[:], [[P * W, 2 * M], [2 * W, 2], [1, W]],
                            extra_offset=W),
                    op=ALU.add)
                U = spool.tile([128, 2 * M * 10], f32, tag=f"U_{pi}")
                nc.vector.tensor_tensor(
                    out=_ap(U[:], [[W, 2 * M], [1, W]]),
                    in0=_ap(U2[:], [[2 * W, 2 * M], [1, W]]),
                    in1=_ap(U2[:], [[2 * W, 2 * M], [1, W]], extra_offset=W),
                    op=ALU.add)
                # fold softmax normalization into u; cast to DT; park in
                # slot qt with layout [j][lv][m] (j-major).
                nc.vector.tensor_tensor(
                    out=_ap(UBslot[:], [[1, 2 * M], [2 * M, W]],
                            extra_offset=qt * 2 * M * W),
                    in0=_ap(U[:], [[W, 2 * M], [1, W]]),
                    in1=_ap(rr[:], [[0, 2], [1, M], [0, W]]),
                    op=ALU.mult)

        def emit_b2(qt):
            # PR[lv][j][m][d] per pair.  CCE path: expand u into PR, then the
            # indirect gather multiplies V rows in on the DMA compute engine.
            PR8 = spool.tile([128, 2 * 8 * M * DH], DT, tag="PR8")
            PR10 = spool.tile([128, 2 * 10 * M * DH], DT, tag="PR10")
            if use_cce:
                for pi, pair in enumerate(PAIRS):
                    W = WCONF[pair[0]]
                    PRt = (PR8, PR10)[pi]
                    UBslot = (UB8s, UB10s)[pi]
                    for lv, l in enumerate(pair):
                        ue = eng_of(ue_eng_s[l])
                        dst = _ap(PRt[:], [[M * DH, W], [DH, M], [1, DH]],
                                  extra_offset=lv * W * M * DH)
                        src = _ap(UBslot[:], [[2 * M, W], [1, M], [0, DH]],
                                  extra_offset=qt * 2 * M * W + lv * M)
                        if ue is nc.scalar:
                            nc.scalar.copy(out=dst, in_=src)
                        else:
                            ue.tensor_copy(out=dst, in_=src)
                for pi, pair in enumerate(PAIRS):
                    W = WCONF[pair[0]]
                    PRt = (PR8, PR10)[pi]
                    for lv, l in enumerate(pair):
                        nc.gpsimd.indirect_dma_start(
                            out=PRt[:, lv * W * DM:(lv + 1) * W * DM],
                            out_offset=None,
                            in_=vp[:],
                            in_offset=bass.IndirectOffsetOnAxis(
                                ap=IDXs[:, qt * L + 2 * pi + lv:
                                        qt * L + 2 * pi + lv + 1], axis=0),
                            oob_is_err=False,
                            compute_op=ALU.mult,
                        )
            else:
                GT8 = gpool.tile([128, 2 * 8 * DM], DT, tag="G8")
                GT10 = gpool.tile([128, 2 * 10 * DM], DT, tag="G10")
                for pi, (Gt, W) in enumerate(((GT8, 8), (GT10, 10))):
                    for lv in range(2):
                        nc.gpsimd.indirect_dma_start(
                            out=Gt[:, lv * W * DM:(lv + 1) * W * DM],
                            out_offset=None,
                            in_=vp[:],
                            in_offset=bass.IndirectOffsetOnAxis(
                                ap=IDXs[:, qt * L + 2 * pi + lv:
                                        qt * L + 2 * pi + lv + 1], axis=0),
                            oob_is_err=False,
                        )
                for pi, pair in enumerate(PAIRS):
                    W = WCONF[pair[0]]
                    Gt = (GT8, GT10)[pi]
                    PRt = (PR8, PR10)[pi]
                    UBslot = (UB8s, UB10s)[pi]
                    for lv, l in enumerate(pair):
                        eng = nc.gpsimd if str(l) in prod_gps else nc.vector
                        eng.tensor_tensor(
                            out=_ap(PRt[:], [[M * DH, W], [DH, M], [1, DH]],
                                    extra_offset=lv * W * M * DH),
                            in0=_ap(Gt[:], [[M * DH, W], [DH, M], [1, DH]],
                                    extra_offset=lv * W * DM),
                            in1=_ap(UBslot[:], [[2 * M, W], [1, M], [0, DH]],
                                    extra_offset=qt * 2 * M * W + lv * M),
                            op=ALU.mult)

            # j-tree sums per pair: contiguous halves within each lv block
            for pi, pair in enumerate(PAIRS):
                W = WCONF[pair[0]]
                PRt = (PR8, PR10)[pi]
                eng = nc.gpsimd if str(pi) in tree_gps else nc.vector
                if W == 10:     # fold j in {8,9} onto {0,1} first
                    eng.tensor_tensor(
                        out=_ap(PRt[:], [[W * M * DH, 2], [1, 2 * M * DH]]),
                        in0=_ap(PRt[:], [[W * M * DH, 2], [1, 2 * M * DH]]),
                        in1=_ap(PRt[:], [[W * M * DH, 2], [1, 2 * M * DH]],
                                extra_offset=8 * M * DH),
                        op=ALU.add)
                w = 8
                while w > 1:
                    h = w // 2
                    eng.tensor_tensor(
                        out=_ap(PRt[:], [[W * M * DH, 2], [1, h * M * DH]]),
                        in0=_ap(PRt[:], [[W * M * DH, 2], [1, h * M * DH]]),
                        in1=_ap(PRt[:], [[W * M * DH, 2], [1, h * M * DH]],
                                extra_offset=h * M * DH),
                        op=ALU.add)
                    w = h

            # sum levels: OUT = (PR8_l0 + PR8_l2) + (PR10_l1 + PR10_l3)
            T8 = spool.tile([128, DM], f32, tag="T8")
            nc.vector.tensor_tensor(
                out=T8[:], in0=PR8[:, 0:DM],
                in1=_ap(PR8[:], [[1, DM]], extra_offset=8 * M * DH),
                op=ALU.add)
            T10 = spool.tile([128, DM], f32, tag="T10")
            nc.vector.tensor_tensor(
                out=T10[:], in0=PR10[:, 0:DM],
                in1=_ap(PR10[:], [[1, DM]], extra_offset=10 * M * DH),
                op=ALU.add)
            OUTT = spool.tile([128, DM], f32, tag="OUTT")
            nc.vector.tensor_tensor(out=OUTT[:], in0=T8[:], in1=T10[:], op=ALU.add)
            nc.sync.dma_start(out=out_d[qt * 128:(qt + 1) * 128, :], in_=OUTT[:])

        rep = max(1, int(os.environ.get("DEFORM_REPEAT", "1")))
        for _ in range(rep):
            for i in range(NQT + LAG):
                if i < NQT:
                    emit_b1(i)
                if i >= LAG:
                    emit_b2(i - LAG)

    nc.compile()
    return nc


def host_prep(inputs):
    """Build per-core in_maps from full inputs."""
    q = np.ascontiguousarray(inputs["query"], np.float32)
    ref = np.ascontiguousarray(np.asarray(inputs["reference_points"])[..., 0], np.float32)
    vin = np.ascontiguousarray(inputs["input_flatten"], np.float32)
    W_val = np.asarray(inputs["W_val"], np.float32)
    b_val = np.asarray(inputs["b_val"], np.float32)
    W_off = np.asarray(inputs["W_off"], np.float32)
    b_off = np.asarray(inputs["b_off"], np.float32)
    W_attn = np.asarray(inputs["W_attn"], np.float32)
    b_attn = np.asarray(inputs["b_attn"], np.float32)

    wv = np.concatenate([W_val.T, b_val[None, :]], 0)
    wof = np.concatenate([W_off.T, b_off[None, :]], 0)
    wat = np.concatenate([W_attn.T, b_attn[None, :]], 0)

    consts = np.zeros((1, CW), np.float32)
    for l in range(L):
        consts[0, C_TL + l] = LENS[l]
        consts[0, C_TM1L + l] = LENS[l] - 1
        consts[0, C_LST + l] = LSTARTP[l]
    consts[0, C_JROW:C_JROW + 16] = np.arange(16, dtype=np.float32)
    consts[0, C_NEG1] = -1.0

    shared = {"wv": np.ascontiguousarray(wv), "wof": np.ascontiguousarray(wof),
              "wat": np.ascontiguousarray(wat), "consts": consts}
    return [
        {"q": q[n], "ref": ref[n], "vin": vin[n], **shared}
        for n in range(N)
    ]


_NC_CACHE = None


def kernel(**inputs) -> np.ndarray:
    global _NC_CACHE
    if _NC_CACHE is None:
        _NC_CACHE = build_program()
    nc = _NC_CACHE
    in_maps = host_prep(inputs)
    res = run_bass_kernel_spmd(nc, in_maps, list(range(N)))
    return np.stack([res.results[n]["out"] for n in range(N)]).astype(np.float32)


if __name__ == "__main__":
    d = np.load("/root/problem/cached_io.npz")
    inp = {k: d[k] for k in ["query", "reference_points", "input_flatten",
                             "input_temporal_lens", "input_level_start_index",
                             "W_val", "b_val", "W_off", "b_off", "W_attn", "b_attn"]}
    out = kernel(**inp)
    ref = d["ref_out"]
    err = np.abs(out - ref).max()
    print("absmax err:", err, "scale:", np.abs(ref).max(),
          "rel:", err / np.abs(ref).max())
